# revision 42
# baseline (speedup 1.0000x reference)
"""Multi-head attention forward on 8 TRN2 NeuronCores (data-parallel over batch).

Reference computation (B=64, T=197, D=768, H=12, DK=64, fp32):
    q = split_heads(x @ Wq + bq); k = ...; v = ...
    scores = floor((q @ k^T) / 8); attn = softmax(scores); out = attn @ v
    return merge_heads(out) @ Wo + bo

Numerics: floor() before softmax makes the Q/K path sensitive.  q/k
projections run as plain fp16 matmuls (N_TERMS=1) with exact fp32
PSUM accumulation.  Measured rel err vs the fp32 reference: 1.657e-2
(budget 2e-2) — bitwise-reproducible across runs (deterministic
inputs, deterministic PE accumulation order) and equal to the numpy
simulation of the same scheme, so the margin is not subject to
run-to-run noise.  N_TERMS=2 (W split hi+lo: + x16 @ W_lo, rel err
1.48e-2, +22us) and N_TERMS=3 (+ xlo @ W_hi, rel err 1.2e-3, +45us)
are kept as fallbacks.  The scores matmul is native fp32, two heads
row-packed via tile_position (packed pairs execute concurrently on
the PE).  The V path (v proj, attn@v, out proj) runs in plain fp16.

Layout: x is transposed + fp16-cast + partition-packed on the HOST
(x16p[p, k*1584+j] = x16[j, k*128+p]) and weights partition-packed
(w[p, k*768+c] = W[k*128+p, c]) so every DMA descriptor is a 9-19KB
contiguous run — the on-chip transpose phase and its 256B-descriptor
DMA-transpose storm are gone entirely (input DMA ~7us, was ~50us).

Schedule: one software-pipelined instruction stream so the PE never
idles (idle >3.4us re-throttles the PE clock to 1.2GHz via HAM).
Projections run in 4 column chunks of 394 rows = one batch pair each.
Attention for pair p (scores -> floor (DVE magic-number round) -> Exp
(ScalarE, -MAGIC folded into the bias) -> attn@v -> normalize -> out
proj) is woven into projection chunk p+1's matmul stream; v-proj units
(dependency-free) pad the latency-sensitive spots (reciprocal ->
broadcast).  Pair 3's attention is layered into iteration 3 as soon
as each head-pair's q/k tiles land, so the tail is short.  attn@v
right-appends a ones column per head (v stride 65) so the softmax
denominator falls out of the same matmul; denominators are gathered
at partitions 0/32 (col block = head pair), reciprocal in two batched
halves (after hp3 / after hp5), then ONE K=33 selector matmul per
head pair broadcasts both reciprocal rows (out rows 0-63 <- partition
0, rows 64-127 <- partition 32; the dn ring buffers are pre-filled
with 1.0 so selector-weight-0 rows stream finite values), DVE
multiply.  PE warmup matmuls + Exp-table preload run during the
initial DMA wait.  PSUM: main 6-bank ring + 2-bank broadcast ring
(decouples broadcasts from Scalar-freed main-ring tenants).

Ring-buffer backlogs (e_t etc.) are sized so WAR reuse never creates
a cross-engine semaphore cycle: an exp writing e_t slot N+16 waits on
the attn@v matmuls of slot N, which must already be issued.
"""

import numpy as np

B, T, D, H, DK = 64, 197, 768, 12, 64
NCORES = 8
BL = B // NCORES          # 8 batch elements per core
R = BL * T                # 1576 rows per core
RPAD = 1584               # row count padded (keeps host packing regular)
ND = D // 128             # 6 chunks of 128 along D
NC4 = 4                   # proj col chunks (one batch pair each)
CW = R // NC4             # 394 = 2*T
HV = DK + 1               # 65: per-head v stride (ones column at 64)
KEYCHUNKS = [(0, 128), (128, 69)]
MAGIC = float(3 * 2 ** 22)  # 1.5*2^23: x-0.5+MAGIC stays in [2^23,2^24), ulp=1
N_TERMS = 1               # 1: q/k = x16@W16; 2: W split hi+lo; 3: + xlo@W_hi

_CACHE = {}


def _build(has_bias, n_terms):
    import concourse.bacc as bacc
    import concourse.mybir as mybir
    import concourse.tile as tile

    f32 = mybir.dt.float32
    f16 = mybir.dt.float16
    AF = mybir.ActivationFunctionType
    OP = mybir.AluOpType

    nc = bacc.Bacc("TRN2", target_bir_lowering=False, debug=False,
                   num_devices=NCORES)

    x16_d = nc.dram_tensor("x16p", [128, ND * RPAD], f16,
                           kind="ExternalInput").ap()
    if n_terms == 3:
        xlo_d = nc.dram_tensor("xlop", [128, ND * RPAD], f16,
                               kind="ExternalInput").ap()
    wnames = (("wq_hi", "wk_hi", "wv", "wo") if n_terms == 1 else
              ("wq_hi", "wq_lo", "wk_hi", "wk_lo", "wv", "wo"))
    w_d = {}
    for nm in wnames:
        w_d[nm] = nc.dram_tensor(nm, [128, ND * D], f16,
                                 kind="ExternalInput").ap()
    if has_bias:
        b_d = {nm: nc.dram_tensor(nm, [1, D], f16, kind="ExternalInput").ap()
               for nm in ("bq", "bk", "bv", "bo")}
    out_d = nc.dram_tensor("out", [R, D], f32, kind="ExternalOutput").ap()

    with tile.TileContext(nc) as tc:
        with tc.tile_pool(name="static", bufs=1) as Ps, \
             tc.tile_pool(name="work", bufs=1) as Pw, \
             tc.tile_pool(name="psum", bufs=8, space="PSUM") as Pp:

            def ptile(nm):
                return Pp.tile([128, CW], f32, name=nm, tag="ps", bufs=6,
                               uniquify=True)

            def bctile():
                # own two-bank ring: broadcasts then only wait on their
                # own previous consumers (DVE mult), not main-ring tenants
                return Pp.tile([128, CW], f32, name="bc", tag="bcp", bufs=2,
                               uniquify=True)

            xall = Ps.tile([128, ND * RPAD], f16, name="xall")
            if n_terms == 3:
                xloall = Ps.tile([128, ND * RPAD], f16, name="xloall")
            wsb = {nm: Ps.tile([128, ND * D], f16, name=nm) for nm in wnames}
            # v16e[2b+kc][keys<=128, 12*65]; col h*65+64 holds ones
            v16e = [Ps.tile([128, H * HV], f16, name=f"v16e_{i}")
                    for i in range(2 * BL)]
            ones_row = Ps.tile([128, CW], f16, name="ones_row")
            negmagic = Ps.tile([128, 1], f32, name="negmagic")
            prime = Ps.tile([1, 1], f16, name="prime")
            # broadcast selector: out rows 0-63 <- rhs row 0 (partition pb),
            # rows 64-127 <- rhs row 32 (partition pb+32), in ONE K=33 matmul
            sel = Ps.tile([128, 128], f16, name="sel")
            if has_bias:
                bsb = {nm: Ps.tile([1, D], f16, name=f"{nm}_sb")
                       for nm in ("bq", "bk", "bv", "bo")}

            def xs(k, c0, ln):
                return xall[:, k * RPAD + c0:k * RPAD + c0 + ln]

            def xls(k, c0, ln):
                return xloall[:, k * RPAD + c0:k * RPAD + c0 + ln]

            def ws(nm, k, c0, ln):
                return wsb[nm][:, k * D + c0:k * D + c0 + ln]

            # ---- no-DMA-dependency setup: memsets, engine warmups ----
            nc.vector.memset(ones_row, 1.0)
            nc.vector.memset(negmagic, -MAGIC)
            nc.vector.memset(sel, 0.0)
            nc.vector.memset(sel[0:1, 0:64], 1.0)
            nc.vector.memset(sel[32:33, 64:128], 1.0)
            # pre-fill the dn ring buffers with 1.0: the K=33 broadcast
            # matmul streams rows 1-31 (selector weight 0) — they must be
            # finite or 0*inf => NaN.  Ring reuse preserves the fill since
            # dn copies only ever write rows 0 and 32.
            for i in range(3):
                dmy = Pw.tile([128, 4 * T], f32, name="dnA", tag="dnA",
                              bufs=3, uniquify=True)
                nc.vector.memset(dmy[:33, :], 1.0)
                dmy = Pw.tile([128, 2 * T], f32, name="dnB", tag="dnB",
                              bufs=3, uniquify=True)
                nc.vector.memset(dmy[:33, :], 1.0)
            for i in range(2 * BL):
                onescol = v16e[i].rearrange("p (h c) -> p h c",
                                            c=HV)[:, :, DK:DK + 1]
                nc.gpsimd.memset(onescol, 1.0)
            # Exp table preload on ScalarE (one-time 1.3us table load)
            nc.scalar.activation(prime, ones_row[:1, :1], AF.Exp,
                                 bias=negmagic[:1, :1])
            # PE warmup: keep HAM at full clock until real work arrives
            for i in range(30):
                wu = ptile("wu")
                nc.tensor.matmul(wu, ones_row[:, :128], ones_row,
                                 start=True, stop=True)

            # ---- DMAs (all large contiguous descriptors) ----
            # x split by column range in consumption order: chunk 0
            # (cols < 400) gates the first projections on just 0.62MB
            x3 = xall.rearrange("p (k j) -> p k j", k=ND)
            xd3 = x16_d.rearrange("p (k j) -> p k j", k=ND)
            nc.sync.dma_start(x3[:, :, :400], xd3[:, :, :400])
            nc.sync.dma_start(wsb["wq_hi"], w_d["wq_hi"])
            if n_terms >= 2:
                nc.sync.dma_start(wsb["wq_lo"], w_d["wq_lo"])
            nc.sync.dma_start(x3[:, :, 400:800], xd3[:, :, 400:800])
            nc.sync.dma_start(wsb["wk_hi"], w_d["wk_hi"])
            if n_terms >= 2:
                nc.sync.dma_start(wsb["wk_lo"], w_d["wk_lo"])
            nc.sync.dma_start(x3[:, :, 800:], xd3[:, :, 800:])
            if n_terms == 3:
                nc.sync.dma_start(xloall, xlo_d)
            nc.sync.dma_start(wsb["wv"], w_d["wv"])
            nc.sync.dma_start(wsb["wo"], w_d["wo"])
            if has_bias:
                for nm in ("bq", "bk", "bv", "bo"):
                    nc.sync.dma_start(bsb[nm], b_d[nm])

            # ---- stage helpers (each call ISSUES instructions) ----
            qT = {}   # (proj, c, n) -> sbuf tile [128, CW] f32
            eTs = {}  # (b, hp) -> [e_t hl0, e_t hl1]
            otfs = {}  # (b, hp) -> otf tile
            oT16s = {}  # (b, hp) -> oT16 tile
            dns = {}
            rd16s = {}

            def qk_tile(c, proj, n):
                whi, wlo, b_nm = (("wq_hi", "wq_lo", "bq") if proj == "q"
                                  else ("wk_hi", "wk_lo", "bk"))
                c0 = c * CW
                pp = ptile("pp")
                for k in range(ND):
                    last = (k == ND - 1 and n_terms == 1 and not has_bias)
                    nc.tensor.matmul(pp, ws(whi, k, n * 128, 128),
                                     xs(k, c0, CW), start=(k == 0),
                                     stop=last)
                for k in range(ND if n_terms >= 2 else 0):
                    last = (k == ND - 1 and n_terms == 2 and not has_bias)
                    nc.tensor.matmul(pp, ws(wlo, k, n * 128, 128),
                                     xs(k, c0, CW), start=False, stop=last)
                if n_terms == 3:
                    for k in range(ND):
                        last = (k == ND - 1 and not has_bias)
                        nc.tensor.matmul(pp, ws(whi, k, n * 128, 128),
                                         xls(k, c0, CW),
                                         start=False, stop=last)
                if has_bias:
                    nc.tensor.matmul(pp, bsb[b_nm][:1, n * 128:n * 128 + 128],
                                     ones_row[:1, :CW],
                                     start=False, stop=True)
                dst = Pw.tile([128, CW], f32, name=f"{proj}T", tag=f"{proj}T",
                              bufs=12, uniquify=True)
                nc.scalar.activation(dst, pp, AF.Copy)
                qT[(proj, c, n)] = dst

            def vp_unit(b, j):
                kc, half = j // 2, j % 2
                koff, klen = KEYCHUNKS[kc]
                base = b * T
                c0 = half * 384
                vp = ptile("vp")
                vps = vp[:klen, :384]
                for d in range(ND):
                    nc.tensor.matmul(
                        vps, xs(d, base + koff, klen), ws("wv", d, c0, 384),
                        start=(d == 0),
                        stop=(d == ND - 1 and not has_bias))
                if has_bias:
                    nc.tensor.matmul(vps, ones_row[:1, :klen],
                                     bsb["bv"][:1, c0:c0 + 384],
                                     start=False, stop=True)
                dst = v16e[2 * b + kc]
                dst3 = dst[:klen, :].rearrange("p (h c) -> p h c",
                                               c=HV)[:, :, 0:DK]
                nc.scalar.activation(
                    dst3[:, half * 6:(half + 1) * 6, :],
                    vps.rearrange("p (h c) -> p h c", c=DK), AF.Copy)

            def sc_unit(b, hp):
                c = b // 2
                qoff = (b % 2) * T
                eT = []
                for hl in range(2):
                    pb = 64 * hl
                    sc = ptile("sc")
                    for kc, (koff, klen) in enumerate(KEYCHUNKS):
                        nc.tensor.matmul(
                            sc[:klen, kc * T:(kc + 1) * T],
                            qT[("k", c, hp)][pb:pb + 64,
                                             qoff + koff:qoff + koff + klen],
                            qT[("q", c, hp)][pb:pb + 64, qoff:qoff + T],
                            start=True, stop=True, tile_position=(pb, 0))
                    fl = Pw.tile([128, 2 * T], f32, name="fl", tag="fl",
                                 bufs=5, uniquify=True)
                    nc.vector.tensor_scalar(fl, sc, -0.5, MAGIC,
                                            OP.add, OP.add)
                    e_t = Pw.tile([128, 2 * T], f16, name="e_t", tag="eT",
                                  bufs=16, uniquify=True)
                    nc.scalar.activation(e_t, fl, AF.Exp,
                                         bias=negmagic[:, :1])
                    eT.append(e_t)
                eTs[(b, hp)] = eT

            def av_unit(b, hp):
                eT = eTs.pop((b, hp))
                if hp == 0:
                    # denominator gather split in two so the reciprocal can
                    # start after hp 0-3 (heads 0-7) instead of after all 12.
                    # head pair hp lands at partitions (0, 32), col block hp
                    dns[b] = (
                        Pw.tile([128, 4 * T], f32, name="dnA", tag="dnA",
                                bufs=3, uniquify=True),
                        Pw.tile([128, 2 * T], f32, name="dnB", tag="dnB",
                                bufs=3, uniquify=True))
                dnA, dnB = dns[b]
                otf = Pw.tile([128, T], f32, name="otf", tag="otf",
                              bufs=12, uniquify=True)
                op_ = ptile("oT")
                for hl in range(2):
                    h = 2 * hp + hl
                    for kc, (koff, klen) in enumerate(KEYCHUNKS):
                        nc.tensor.matmul(
                            op_[0:HV, hl * T:(hl + 1) * T],
                            v16e[2 * b + kc][:klen, h * HV:(h + 1) * HV],
                            eT[hl][:klen, kc * T:(kc + 1) * T],
                            start=(kc == 0), stop=(kc == len(KEYCHUNKS) - 1))
                    pbase = 32 * hl
                    dn, cb = (dnA, hp * T) if hp < 4 else (dnB, (hp - 4) * T)
                    nc.vector.tensor_copy(dn[pbase:pbase + 1, cb:cb + T],
                                          op_[64:65, hl * T:(hl + 1) * T])
                    if hl == 0:
                        nc.scalar.activation(otf[0:64, :], op_[0:64, :T],
                                             AF.Copy)
                    else:
                        nc.vector.tensor_copy(otf[64:128, :],
                                              op_[0:64, T:2 * T])
                otfs[(b, hp)] = otf

            def recip_unit(b, part):
                dnA, dnB = dns[b]
                # the fp16 cast runs on idle GpSimd in steady state, but on
                # DVE for the tail pair where the rd16->bcast latency matters
                cp = nc.vector.tensor_copy if b >= 6 else nc.gpsimd.tensor_copy
                if part == 0:
                    rdf = Pw.tile([128, 4 * T], f32, name="rdfA", tag="rdfA",
                                  bufs=1, uniquify=True)
                    rd16 = Pw.tile([128, 4 * T], f16, name="rdA", tag="rdA",
                                   bufs=3, uniquify=True)
                    nc.vector.reciprocal_approx_fast(rdf[:33, :],
                                                     dnA[:33, :])
                    cp(rd16[:33, :], rdf[:33, :])
                    rd16s[b] = [rd16, None]
                else:
                    rdf = Pw.tile([128, 2 * T], f32, name="rdfB", tag="rdfB",
                                  bufs=1, uniquify=True)
                    rd16 = Pw.tile([128, 2 * T], f16, name="rdB", tag="rdB",
                                   bufs=3, uniquify=True)
                    nc.vector.reciprocal_approx_fast(rdf[:33, :],
                                                     dnB[:33, :])
                    cp(rd16[:33, :], rdf[:33, :])
                    rd16s[b][1] = rd16
                    del dns[b]

            def bc_unit(b, hp):
                bc = bctile()
                rd16, cb = ((rd16s[b][0], hp * T) if hp < 4
                            else (rd16s[b][1], (hp - 4) * T))
                nc.tensor.matmul(
                    bc[:, :T], sel[0:33, :128], rd16[0:33, cb:cb + T],
                    start=True, stop=True)
                oT16 = Pw.tile([128, T], f16, name="oT16", tag="oT16",
                               bufs=14, uniquify=True)
                nc.vector.tensor_tensor(oT16, otfs.pop((b, hp)), bc[:, :T],
                                        OP.mult)
                oT16s[(b, hp)] = oT16

            def op_unit(b, rc):
                base = b * T
                roff, rn = (0, 128) if rc == 0 else (128, T - 128)
                fs = Pw.tile([128, D], f32, name="fs", tag="fs",
                             bufs=3, uniquify=True)
                for half in range(2):
                    c0 = half * 384
                    fp_ = ptile("fp")
                    for d in range(ND):
                        nc.tensor.matmul(
                            fp_[:rn, :384],
                            oT16s[(b, d)][:, roff:roff + rn],
                            ws("wo", d, c0, 384),
                            start=(d == 0),
                            stop=(d == ND - 1 and not has_bias))
                    if has_bias:
                        nc.tensor.matmul(fp_[:rn, :384], ones_row[:1, :rn],
                                         bsb["bo"][:1, c0:c0 + 384],
                                         start=False, stop=True)
                    # Scalar runs 86-91% busy in the c3/tail region (exps);
                    # route the out-proj staging copies there to Vector
                    if b >= 4:
                        nc.vector.tensor_copy(fs[:rn, c0:c0 + 384],
                                              fp_[:rn, :384])
                    else:
                        nc.scalar.activation(fs[:rn, c0:c0 + 384],
                                             fp_[:rn, :384], AF.Copy)
                    nc.sync.dma_start(
                        out_d[base + roff:base + roff + rn, c0:c0 + 384],
                        fs[:rn, c0:c0 + 384])
                if rc == 1:
                    for d in range(ND):
                        del oT16s[(b, d)]

            # ---- pipelined program ----
            # c0 prologue: projections chunk 0 + v for pair 0
            for n in range(ND):
                qk_tile(0, "q", n)
            for n in range(ND):
                qk_tile(0, "k", n)
            for j in range(4):
                vp_unit(0, j)
            for j in range(4):
                vp_unit(1, j)

            # iterations c=1,2: attention(pair c-1) woven into chunk c
            # interleaved q/k tile order: after tiles 2n,2n+1 the score
            # units for head-pair n of the NEXT pair are unblocked
            qkt = [(p, n) for n in range(ND) for p in ("q", "k")]
            for c in range(1, 3):
                B0, B1 = 2 * (c - 1), 2 * (c - 1) + 1
                C0, C1 = 2 * c, 2 * c + 1

                sc_unit(B0, 0); sc_unit(B0, 1)
                qk_tile(c, *qkt[0])
                sc_unit(B0, 2); av_unit(B0, 0)
                qk_tile(c, *qkt[1])
                sc_unit(B0, 3); av_unit(B0, 1)
                qk_tile(c, *qkt[2])
                sc_unit(B0, 4); av_unit(B0, 2)
                qk_tile(c, *qkt[3])
                sc_unit(B0, 5); av_unit(B0, 3)
                recip_unit(B0, 0)
                qk_tile(c, *qkt[4])
                av_unit(B0, 4); av_unit(B0, 5)
                recip_unit(B0, 1)
                qk_tile(c, *qkt[5])
                sc_unit(B1, 0); sc_unit(B1, 1)
                qk_tile(c, *qkt[6])
                sc_unit(B1, 2); av_unit(B1, 0)
                bc_unit(B0, 0); bc_unit(B0, 1)
                qk_tile(c, *qkt[7])
                sc_unit(B1, 3); av_unit(B1, 1)
                bc_unit(B0, 2); bc_unit(B0, 3)
                qk_tile(c, *qkt[8])
                sc_unit(B1, 4); av_unit(B1, 2)
                bc_unit(B0, 4); bc_unit(B0, 5)
                qk_tile(c, *qkt[9])
                sc_unit(B1, 5); av_unit(B1, 3)
                recip_unit(B1, 0)
                op_unit(B0, 0)
                qk_tile(c, *qkt[10])
                av_unit(B1, 4)
                op_unit(B0, 1)
                qk_tile(c, *qkt[11])
                av_unit(B1, 5)
                recip_unit(B1, 1)
                vp_unit(C0, 0); vp_unit(C0, 1)
                bc_unit(B1, 0); bc_unit(B1, 1); bc_unit(B1, 2)
                vp_unit(C0, 2); vp_unit(C0, 3)
                bc_unit(B1, 3); bc_unit(B1, 4); bc_unit(B1, 5)
                vp_unit(C1, 0); vp_unit(C1, 1)
                op_unit(B1, 0)
                vp_unit(C1, 2); vp_unit(C1, 3)
                op_unit(B1, 1)

            # iteration 3 + tail, fully merged: att(pair 2) woven into
            # chunk 3, with pair-3 attention layered in as soon as each
            # head-pair's q/k tiles land, so the PE stays dense to the end.
            # (sc(6/7, n) needs chunk-3 tiles q_n AND k_n = qkt[2n+1];
            # e_t ring backlog must stay <= bufs-2 to avoid WAR cycles.)
            sc_unit(4, 0); sc_unit(4, 1)
            qk_tile(3, *qkt[0])
            sc_unit(4, 2); av_unit(4, 0)
            qk_tile(3, *qkt[1])
            sc_unit(4, 3); av_unit(4, 1)
            qk_tile(3, *qkt[2])
            sc_unit(4, 4); av_unit(4, 2)
            qk_tile(3, *qkt[3])
            sc_unit(4, 5); av_unit(4, 3)
            recip_unit(4, 0)
            qk_tile(3, *qkt[4])
            av_unit(4, 4); av_unit(4, 5)
            recip_unit(4, 1)
            qk_tile(3, *qkt[5])
            sc_unit(5, 0); sc_unit(5, 1)
            qk_tile(3, *qkt[6])
            sc_unit(5, 2); av_unit(5, 0); sc_unit(6, 0)
            bc_unit(4, 0); bc_unit(4, 1); bc_unit(4, 2)
            qk_tile(3, *qkt[7])
            sc_unit(5, 3); av_unit(5, 1); sc_unit(6, 1)
            bc_unit(4, 3); bc_unit(4, 4); bc_unit(4, 5)
            vp_unit(6, 0); vp_unit(6, 1)
            qk_tile(3, *qkt[8])
            sc_unit(5, 4); av_unit(5, 2); sc_unit(6, 2)
            vp_unit(6, 2); vp_unit(6, 3)
            qk_tile(3, *qkt[9])
            sc_unit(5, 5); av_unit(5, 3); av_unit(6, 0)
            recip_unit(5, 0)
            op_unit(4, 0)
            qk_tile(3, *qkt[10])
            av_unit(5, 4); av_unit(5, 5); av_unit(6, 1)
            recip_unit(5, 1)
            qk_tile(3, *qkt[11])
            sc_unit(6, 3); sc_unit(6, 4)
            vp_unit(7, 0); vp_unit(7, 1); vp_unit(7, 2); vp_unit(7, 3)
            bc_unit(5, 0); bc_unit(5, 1); bc_unit(5, 2)
            av_unit(6, 2); sc_unit(6, 5)
            op_unit(4, 1)
            bc_unit(5, 3); bc_unit(5, 4); bc_unit(5, 5)
            av_unit(6, 3); sc_unit(7, 0)
            recip_unit(6, 0)
            op_unit(5, 0)
            av_unit(6, 4); sc_unit(7, 1)
            op_unit(5, 1)
            av_unit(6, 5); sc_unit(7, 2)
            recip_unit(6, 1)
            av_unit(7, 0); sc_unit(7, 3)
            bc_unit(6, 0); bc_unit(6, 1); bc_unit(6, 2)
            av_unit(7, 1); sc_unit(7, 4)
            bc_unit(6, 3); bc_unit(6, 4); bc_unit(6, 5)
            av_unit(7, 2); sc_unit(7, 5)
            av_unit(7, 3)
            recip_unit(7, 0)
            op_unit(6, 0)
            av_unit(7, 4); av_unit(7, 5)
            recip_unit(7, 1)
            op_unit(6, 1)
            bc_unit(7, 0); bc_unit(7, 1); bc_unit(7, 2)
            bc_unit(7, 3); bc_unit(7, 4); bc_unit(7, 5)
            op_unit(7, 0)
            op_unit(7, 1)

    nc.compile()
    return nc


def _split16(a):
    hi = a.astype(np.float16)
    lo = (a - hi.astype(np.float32)).astype(np.float16)
    return hi, lo


def _pack_w(a16):
    # [768, N] fp16 -> [128, 6*N]: out[p, k*N+c] = a16[k*128+p, c]
    N = a16.shape[1]
    return np.ascontiguousarray(
        a16.reshape(ND, 128, N).transpose(1, 0, 2).reshape(128, ND * N))


def _prep_weights(Wq, bq, Wk, bk, Wv, bv, Wo, bo, has_bias):
    f32 = np.float32
    wq = np.asarray(Wq, f32) * f32(0.125)
    wk = np.asarray(Wk, f32)
    wq_hi, wq_lo = _split16(wq)
    wk_hi, wk_lo = _split16(wk)
    w = {
        "wq_hi": _pack_w(wq_hi),
        "wk_hi": _pack_w(wk_hi),
        "wv": _pack_w(np.asarray(Wv, f32).astype(np.float16)),
        "wo": _pack_w(np.asarray(Wo, f32).astype(np.float16)),
    }
    if N_TERMS >= 2:
        w["wq_lo"] = _pack_w(wq_lo)
        w["wk_lo"] = _pack_w(wk_lo)
    if has_bias:
        w["bq"] = (np.asarray(bq, f32) * f32(0.125)).astype(
            np.float16).reshape(1, D)
        w["bk"] = np.asarray(bk, f32).astype(np.float16).reshape(1, D)
        w["bv"] = np.asarray(bv, f32).astype(np.float16).reshape(1, D)
        w["bo"] = np.asarray(bo, f32).astype(np.float16).reshape(1, D)
    return w


def _pack_x(xc16):
    # [R, 768] fp16 -> [128, 6*RPAD]: out[p, k*RPAD+j] = x16[j, k*128+p]
    xt = np.zeros((D, RPAD), np.float16)
    xt[:, :R] = xc16.T
    return np.ascontiguousarray(
        xt.reshape(ND, 128, RPAD).transpose(1, 0, 2).reshape(128, ND * RPAD))


def _make_in_maps(x, w):
    x = np.asarray(x, np.float32)
    in_maps = []
    for c in range(NCORES):
        m = dict(w)
        xc = x[c * BL:(c + 1) * BL].reshape(R, D)
        x16 = xc.astype(np.float16)
        m["x16p"] = _pack_x(x16)
        if N_TERMS == 3:
            m["xlop"] = _pack_x(
                (xc - x16.astype(np.float32)).astype(np.float16))
        in_maps.append(m)
    return in_maps


def kernel(x, Wq, bq, Wk, bk, Wv, bv, Wo, bo):
    from concourse import bass_utils

    has_bias = any(float(np.abs(np.asarray(v)).max()) != 0.0
                   for v in (bq, bk, bv, bo))
    key = ("nc", has_bias, N_TERMS)
    if key not in _CACHE:
        _CACHE[key] = _build(has_bias, N_TERMS)
    nc = _CACHE[key]

    w = _prep_weights(Wq, bq, Wk, bk, Wv, bv, Wo, bo, has_bias)
    in_maps = _make_in_maps(x, w)

    res = bass_utils.run_bass_kernel_spmd(nc, in_maps, list(range(NCORES)))
    out = np.concatenate(
        [res.results[c]["out"].reshape(BL, T, D) for c in range(NCORES)],
        axis=0)
    return out.astype(np.float32)


# revision 43
# speedup vs baseline: 1.0250x; 1.0250x over previous
"""Multi-head attention forward on 8 TRN2 NeuronCores (data-parallel over batch).

Reference computation (B=64, T=197, D=768, H=12, DK=64, fp32):
    q = split_heads(x @ Wq + bq); k = ...; v = ...
    scores = floor((q @ k^T) / 8); attn = softmax(scores); out = attn @ v
    return merge_heads(out) @ Wo + bo

Numerics: floor() before softmax makes the Q/K path sensitive.  q/k
projections run as plain fp16 matmuls (N_TERMS=1) with exact fp32
PSUM accumulation.  Measured rel err vs the fp32 reference: 1.657e-2
(budget 2e-2) — bitwise-reproducible across runs (deterministic
inputs, deterministic PE accumulation order) and equal to the numpy
simulation of the same scheme, so the margin is not subject to
run-to-run noise.  N_TERMS=2 (W split hi+lo: + x16 @ W_lo, rel err
1.48e-2, +22us) and N_TERMS=3 (+ xlo @ W_hi, rel err 1.2e-3, +45us)
are kept as fallbacks.  The scores matmul is native fp32, two heads
row-packed via tile_position (packed pairs execute concurrently on
the PE).  The V path (v proj, attn@v, out proj) runs in plain fp16.

Layout: x is transposed + fp16-cast + partition-packed on the HOST
(x16p[p, k*1584+j] = x16[j, k*128+p]) and weights partition-packed
(w[p, k*768+c] = W[k*128+p, c]) so every DMA descriptor is a 9-19KB
contiguous run — the on-chip transpose phase and its 256B-descriptor
DMA-transpose storm are gone entirely (input DMA ~7us, was ~50us).

Schedule: one software-pipelined instruction stream so the PE never
idles (idle >3.4us re-throttles the PE clock to 1.2GHz via HAM).
Projections run in 4 column chunks of 394 rows = one batch pair each.
Attention for pair p (scores -> floor (DVE magic-number round) -> Exp
(ScalarE, -MAGIC folded into the bias) -> attn@v -> normalize -> out
proj) is woven into projection chunk p+1's matmul stream; v-proj units
(dependency-free) pad the latency-sensitive spots (reciprocal ->
broadcast).  Pair 3's attention is layered into iteration 3 as soon
as each head-pair's q/k tiles land, so the tail is short.  attn@v
right-appends a ones column per head (v stride 65) so the softmax
denominator falls out of the same matmul; denominators are gathered
at partitions 0/32 (col block = head pair), reciprocal in two batched
halves (after hp3 / after hp5), then ONE K=33 selector matmul per
head pair broadcasts both reciprocal rows (out rows 0-63 <- partition
0, rows 64-127 <- partition 32; the dn ring buffers are pre-filled
with 1.0 so selector-weight-0 rows stream finite values), DVE
multiply.  PE warmup matmuls + Exp-table preload run during the
initial DMA wait.  PSUM: main 6-bank ring + 2-bank broadcast ring
(decouples broadcasts from Scalar-freed main-ring tenants).

Ring-buffer backlogs (e_t etc.) are sized so WAR reuse never creates
a cross-engine semaphore cycle: an exp writing e_t slot N+16 waits on
the attn@v matmuls of slot N, which must already be issued.
"""

import numpy as np

B, T, D, H, DK = 64, 197, 768, 12, 64
NCORES = 8
BL = B // NCORES          # 8 batch elements per core
R = BL * T                # 1576 rows per core
RPAD = 1584               # row count padded (keeps host packing regular)
ND = D // 128             # 6 chunks of 128 along D
NC4 = 4                   # proj col chunks (one batch pair each)
CW = R // NC4             # 394 = 2*T
HV = DK + 1               # 65: per-head v stride (ones column at 64)
KEYCHUNKS = [(0, 128), (128, 69)]
MAGIC = float(3 * 2 ** 22)  # 1.5*2^23: x-0.5+MAGIC stays in [2^23,2^24), ulp=1
N_TERMS = 1               # 1: q/k = x16@W16; 2: W split hi+lo; 3: + xlo@W_hi

_CACHE = {}


def _build(has_bias, n_terms):
    import concourse.bacc as bacc
    import concourse.mybir as mybir
    import concourse.tile as tile

    f32 = mybir.dt.float32
    f16 = mybir.dt.float16
    AF = mybir.ActivationFunctionType
    OP = mybir.AluOpType

    nc = bacc.Bacc("TRN2", target_bir_lowering=False, debug=False,
                   num_devices=NCORES)

    x16_d = nc.dram_tensor("x16p", [128, ND * RPAD], f16,
                           kind="ExternalInput").ap()
    if n_terms == 3:
        xlo_d = nc.dram_tensor("xlop", [128, ND * RPAD], f16,
                               kind="ExternalInput").ap()
    wnames = (("wq_hi", "wk_hi", "wv", "wo") if n_terms == 1 else
              ("wq_hi", "wq_lo", "wk_hi", "wk_lo", "wv", "wo"))
    w_d = {}
    for nm in wnames:
        w_d[nm] = nc.dram_tensor(nm, [128, ND * D], f16,
                                 kind="ExternalInput").ap()
    if has_bias:
        b_d = {nm: nc.dram_tensor(nm, [1, D], f16, kind="ExternalInput").ap()
               for nm in ("bq", "bk", "bv", "bo")}
    out_d = nc.dram_tensor("out", [R, D], f32, kind="ExternalOutput").ap()

    with tile.TileContext(nc) as tc:
        with tc.tile_pool(name="static", bufs=1) as Ps, \
             tc.tile_pool(name="work", bufs=1) as Pw, \
             tc.tile_pool(name="psum", bufs=8, space="PSUM") as Pp:

            def ptile(nm):
                return Pp.tile([128, CW], f32, name=nm, tag="ps", bufs=6,
                               uniquify=True)

            def bctile():
                # own two-bank ring: broadcasts then only wait on their
                # own previous consumers (DVE mult), not main-ring tenants
                return Pp.tile([128, CW], f32, name="bc", tag="bcp", bufs=2,
                               uniquify=True)

            xall = Ps.tile([128, ND * RPAD], f16, name="xall")
            if n_terms == 3:
                xloall = Ps.tile([128, ND * RPAD], f16, name="xloall")
            wsb = {nm: Ps.tile([128, ND * D], f16, name=nm) for nm in wnames}
            # v16e[2b+kc][keys<=128, 12*65]; col h*65+64 holds ones
            v16e = [Ps.tile([128, H * HV], f16, name=f"v16e_{i}")
                    for i in range(2 * BL)]
            ones_row = Ps.tile([128, CW], f16, name="ones_row")
            negmagic = Ps.tile([128, 1], f32, name="negmagic")
            prime = Ps.tile([1, 1], f16, name="prime")
            # broadcast selector: out rows 0-63 <- rhs row 0 (partition pb),
            # rows 64-127 <- rhs row 32 (partition pb+32), in ONE K=33 matmul
            sel = Ps.tile([128, 128], f16, name="sel")
            if has_bias:
                bsb = {nm: Ps.tile([1, D], f16, name=f"{nm}_sb")
                       for nm in ("bq", "bk", "bv", "bo")}

            def xs(k, c0, ln):
                return xall[:, k * RPAD + c0:k * RPAD + c0 + ln]

            def xls(k, c0, ln):
                return xloall[:, k * RPAD + c0:k * RPAD + c0 + ln]

            def ws(nm, k, c0, ln):
                return wsb[nm][:, k * D + c0:k * D + c0 + ln]

            # ---- no-DMA-dependency setup: memsets, engine warmups ----
            nc.vector.memset(ones_row, 1.0)
            nc.vector.memset(negmagic, -MAGIC)
            nc.vector.memset(sel, 0.0)
            nc.vector.memset(sel[0:1, 0:64], 1.0)
            nc.vector.memset(sel[32:33, 64:128], 1.0)
            # pre-fill the dn ring buffers with 1.0: the K=33 broadcast
            # matmul streams rows 1-31 (selector weight 0) — they must be
            # finite or 0*inf => NaN.  Ring reuse preserves the fill since
            # dn copies only ever write rows 0 and 32.
            for i in range(3):
                dmy = Pw.tile([128, 4 * T], f32, name="dnA", tag="dnA",
                              bufs=3, uniquify=True)
                nc.vector.memset(dmy[:33, :], 1.0)
                dmy = Pw.tile([128, 2 * T], f32, name="dnB", tag="dnB",
                              bufs=3, uniquify=True)
                nc.vector.memset(dmy[:33, :], 1.0)
            for i in range(2 * BL):
                onescol = v16e[i].rearrange("p (h c) -> p h c",
                                            c=HV)[:, :, DK:DK + 1]
                nc.gpsimd.memset(onescol, 1.0)
            # Exp table preload on ScalarE (one-time 1.3us table load)
            nc.scalar.activation(prime, ones_row[:1, :1], AF.Exp,
                                 bias=negmagic[:1, :1])
            # PE warmup: keep HAM at full clock until real work arrives
            for i in range(30):
                wu = ptile("wu")
                nc.tensor.matmul(wu, ones_row[:, :128], ones_row,
                                 start=True, stop=True)

            # ---- DMAs (all large contiguous descriptors) ----
            # x split by column range in consumption order: chunk 0
            # (cols < 400) gates the first projections on just 0.62MB
            x3 = xall.rearrange("p (k j) -> p k j", k=ND)
            xd3 = x16_d.rearrange("p (k j) -> p k j", k=ND)
            nc.sync.dma_start(x3[:, :, :400], xd3[:, :, :400])
            nc.sync.dma_start(wsb["wq_hi"], w_d["wq_hi"])
            if n_terms >= 2:
                nc.sync.dma_start(wsb["wq_lo"], w_d["wq_lo"])
            nc.sync.dma_start(x3[:, :, 400:800], xd3[:, :, 400:800])
            nc.sync.dma_start(wsb["wk_hi"], w_d["wk_hi"])
            if n_terms >= 2:
                nc.sync.dma_start(wsb["wk_lo"], w_d["wk_lo"])
            nc.sync.dma_start(x3[:, :, 800:], xd3[:, :, 800:])
            if n_terms == 3:
                nc.sync.dma_start(xloall, xlo_d)
            nc.sync.dma_start(wsb["wv"], w_d["wv"])
            nc.sync.dma_start(wsb["wo"], w_d["wo"])
            if has_bias:
                for nm in ("bq", "bk", "bv", "bo"):
                    nc.sync.dma_start(bsb[nm], b_d[nm])

            # ---- stage helpers (each call ISSUES instructions) ----
            qT = {}   # (proj, c, n) -> sbuf tile [128, CW] f32
            eTs = {}  # (b, hp) -> [e_t hl0, e_t hl1]
            otfs = {}  # (b, hp) -> otf tile
            oT16s = {}  # (b, hp) -> oT16 tile
            dns = {}
            rd16s = {}

            def qk_tile(c, proj, n):
                whi, wlo, b_nm = (("wq_hi", "wq_lo", "bq") if proj == "q"
                                  else ("wk_hi", "wk_lo", "bk"))
                c0 = c * CW
                pp = ptile("pp")
                for k in range(ND):
                    last = (k == ND - 1 and n_terms == 1 and not has_bias)
                    nc.tensor.matmul(pp, ws(whi, k, n * 128, 128),
                                     xs(k, c0, CW), start=(k == 0),
                                     stop=last)
                for k in range(ND if n_terms >= 2 else 0):
                    last = (k == ND - 1 and n_terms == 2 and not has_bias)
                    nc.tensor.matmul(pp, ws(wlo, k, n * 128, 128),
                                     xs(k, c0, CW), start=False, stop=last)
                if n_terms == 3:
                    for k in range(ND):
                        last = (k == ND - 1 and not has_bias)
                        nc.tensor.matmul(pp, ws(whi, k, n * 128, 128),
                                         xls(k, c0, CW),
                                         start=False, stop=last)
                if has_bias:
                    nc.tensor.matmul(pp, bsb[b_nm][:1, n * 128:n * 128 + 128],
                                     ones_row[:1, :CW],
                                     start=False, stop=True)
                dst = Pw.tile([128, CW], f32, name=f"{proj}T", tag=f"{proj}T",
                              bufs=12, uniquify=True)
                nc.scalar.activation(dst, pp, AF.Copy)
                qT[(proj, c, n)] = dst

            def vp_unit(b, j):
                kc, half = j // 2, j % 2
                koff, klen = KEYCHUNKS[kc]
                base = b * T
                c0 = half * 384
                vp = ptile("vp")
                vps = vp[:klen, :384]
                for d in range(ND):
                    nc.tensor.matmul(
                        vps, xs(d, base + koff, klen), ws("wv", d, c0, 384),
                        start=(d == 0),
                        stop=(d == ND - 1 and not has_bias))
                if has_bias:
                    nc.tensor.matmul(vps, ones_row[:1, :klen],
                                     bsb["bv"][:1, c0:c0 + 384],
                                     start=False, stop=True)
                dst = v16e[2 * b + kc]
                dst3 = dst[:klen, :].rearrange("p (h c) -> p h c",
                                               c=HV)[:, :, 0:DK]
                nc.scalar.activation(
                    dst3[:, half * 6:(half + 1) * 6, :],
                    vps.rearrange("p (h c) -> p h c", c=DK), AF.Copy)

            def sc_unit(b, hp):
                c = b // 2
                qoff = (b % 2) * T
                eT = []
                for hl in range(2):
                    pb = 64 * hl
                    sc = ptile("sc")
                    for kc, (koff, klen) in enumerate(KEYCHUNKS):
                        nc.tensor.matmul(
                            sc[:klen, kc * T:(kc + 1) * T],
                            qT[("k", c, hp)][pb:pb + 64,
                                             qoff + koff:qoff + koff + klen],
                            qT[("q", c, hp)][pb:pb + 64, qoff:qoff + T],
                            start=True, stop=True, tile_position=(pb, 0))
                    fl = Pw.tile([128, 2 * T], f32, name="fl", tag="fl",
                                 bufs=5, uniquify=True)
                    nc.vector.tensor_scalar(fl, sc, -0.5, MAGIC,
                                            OP.add, OP.add)
                    e_t = Pw.tile([128, 2 * T], f16, name="e_t", tag="eT",
                                  bufs=16, uniquify=True)
                    nc.scalar.activation(e_t, fl, AF.Exp,
                                         bias=negmagic[:, :1])
                    eT.append(e_t)
                eTs[(b, hp)] = eT

            def av_unit(b, hp):
                eT = eTs.pop((b, hp))
                if hp == 0:
                    # denominator gather split in two so the reciprocal can
                    # start after hp 0-3 (heads 0-7) instead of after all 12.
                    # head pair hp lands at partitions (0, 32), col block hp
                    dns[b] = (
                        Pw.tile([128, 4 * T], f32, name="dnA", tag="dnA",
                                bufs=3, uniquify=True),
                        Pw.tile([128, 2 * T], f32, name="dnB", tag="dnB",
                                bufs=3, uniquify=True))
                dnA, dnB = dns[b]
                otf = Pw.tile([128, T], f32, name="otf", tag="otf",
                              bufs=12, uniquify=True)
                op_ = ptile("oT")
                for hl in range(2):
                    h = 2 * hp + hl
                    for kc, (koff, klen) in enumerate(KEYCHUNKS):
                        nc.tensor.matmul(
                            op_[0:HV, hl * T:(hl + 1) * T],
                            v16e[2 * b + kc][:klen, h * HV:(h + 1) * HV],
                            eT[hl][:klen, kc * T:(kc + 1) * T],
                            start=(kc == 0), stop=(kc == len(KEYCHUNKS) - 1))
                    pbase = 32 * hl
                    dn, cb = (dnA, hp * T) if hp < 4 else (dnB, (hp - 4) * T)
                    # tail pairs: keep the Vector queue clear for the
                    # recip->cast->broadcast chain (ScalarE idles there)
                    if b >= 6:
                        nc.scalar.activation(
                            dn[pbase:pbase + 1, cb:cb + T],
                            op_[64:65, hl * T:(hl + 1) * T], AF.Copy)
                    else:
                        nc.vector.tensor_copy(
                            dn[pbase:pbase + 1, cb:cb + T],
                            op_[64:65, hl * T:(hl + 1) * T])
                    if hl == 0:
                        nc.scalar.activation(otf[0:64, :], op_[0:64, :T],
                                             AF.Copy)
                    elif b >= 6:
                        nc.scalar.activation(otf[64:128, :],
                                             op_[0:64, T:2 * T], AF.Copy)
                    else:
                        nc.vector.tensor_copy(otf[64:128, :],
                                              op_[0:64, T:2 * T])
                otfs[(b, hp)] = otf

            def recip_unit(b, part):
                dnA, dnB = dns[b]
                # the fp16 cast runs on idle GpSimd in steady state, but on
                # DVE for the tail pair where the rd16->bcast latency matters
                cp = nc.vector.tensor_copy if b >= 6 else nc.gpsimd.tensor_copy
                if part == 0:
                    rdf = Pw.tile([128, 4 * T], f32, name="rdfA", tag="rdfA",
                                  bufs=1, uniquify=True)
                    rd16 = Pw.tile([128, 4 * T], f16, name="rdA", tag="rdA",
                                   bufs=3, uniquify=True)
                    nc.vector.reciprocal_approx_fast(rdf[:33, :],
                                                     dnA[:33, :])
                    cp(rd16[:33, :], rdf[:33, :])
                    rd16s[b] = [rd16, None]
                else:
                    rdf = Pw.tile([128, 2 * T], f32, name="rdfB", tag="rdfB",
                                  bufs=1, uniquify=True)
                    rd16 = Pw.tile([128, 2 * T], f16, name="rdB", tag="rdB",
                                   bufs=3, uniquify=True)
                    nc.vector.reciprocal_approx_fast(rdf[:33, :],
                                                     dnB[:33, :])
                    cp(rd16[:33, :], rdf[:33, :])
                    rd16s[b][1] = rd16
                    del dns[b]

            def bc_unit(b, hp):
                bc = bctile()
                rd16, cb = ((rd16s[b][0], hp * T) if hp < 4
                            else (rd16s[b][1], (hp - 4) * T))
                nc.tensor.matmul(
                    bc[:, :T], sel[0:33, :128], rd16[0:33, cb:cb + T],
                    start=True, stop=True)
                oT16 = Pw.tile([128, T], f16, name="oT16", tag="oT16",
                               bufs=14, uniquify=True)
                nc.vector.tensor_tensor(oT16, otfs.pop((b, hp)), bc[:, :T],
                                        OP.mult)
                oT16s[(b, hp)] = oT16

            def op_unit(b, rc):
                base = b * T
                roff, rn = (0, 128) if rc == 0 else (128, T - 128)
                fs = Pw.tile([128, D], f32, name="fs", tag="fs",
                             bufs=3, uniquify=True)
                for half in range(2):
                    c0 = half * 384
                    fp_ = ptile("fp")
                    for d in range(ND):
                        nc.tensor.matmul(
                            fp_[:rn, :384],
                            oT16s[(b, d)][:, roff:roff + rn],
                            ws("wo", d, c0, 384),
                            start=(d == 0),
                            stop=(d == ND - 1 and not has_bias))
                    if has_bias:
                        nc.tensor.matmul(fp_[:rn, :384], ones_row[:1, :rn],
                                         bsb["bo"][:1, c0:c0 + 384],
                                         start=False, stop=True)
                    nc.scalar.activation(fs[:rn, c0:c0 + 384],
                                         fp_[:rn, :384], AF.Copy)
                    nc.sync.dma_start(
                        out_d[base + roff:base + roff + rn, c0:c0 + 384],
                        fs[:rn, c0:c0 + 384])
                if rc == 1:
                    for d in range(ND):
                        del oT16s[(b, d)]

            # ---- pipelined program ----
            # c0 prologue: projections chunk 0 + v for pair 0
            for n in range(ND):
                qk_tile(0, "q", n)
            for n in range(ND):
                qk_tile(0, "k", n)
            for j in range(4):
                vp_unit(0, j)
            for j in range(4):
                vp_unit(1, j)

            # iterations c=1,2: attention(pair c-1) woven into chunk c
            # interleaved q/k tile order: after tiles 2n,2n+1 the score
            # units for head-pair n of the NEXT pair are unblocked
            qkt = [(p, n) for n in range(ND) for p in ("q", "k")]
            for c in range(1, 3):
                B0, B1 = 2 * (c - 1), 2 * (c - 1) + 1
                C0, C1 = 2 * c, 2 * c + 1

                sc_unit(B0, 0); sc_unit(B0, 1)
                qk_tile(c, *qkt[0])
                sc_unit(B0, 2); av_unit(B0, 0)
                qk_tile(c, *qkt[1])
                sc_unit(B0, 3); av_unit(B0, 1)
                qk_tile(c, *qkt[2])
                sc_unit(B0, 4); av_unit(B0, 2)
                qk_tile(c, *qkt[3])
                sc_unit(B0, 5); av_unit(B0, 3)
                recip_unit(B0, 0)
                qk_tile(c, *qkt[4])
                av_unit(B0, 4); av_unit(B0, 5)
                recip_unit(B0, 1)
                qk_tile(c, *qkt[5])
                sc_unit(B1, 0); sc_unit(B1, 1)
                qk_tile(c, *qkt[6])
                sc_unit(B1, 2); av_unit(B1, 0)
                bc_unit(B0, 0); bc_unit(B0, 1)
                qk_tile(c, *qkt[7])
                sc_unit(B1, 3); av_unit(B1, 1)
                bc_unit(B0, 2); bc_unit(B0, 3)
                qk_tile(c, *qkt[8])
                sc_unit(B1, 4); av_unit(B1, 2)
                bc_unit(B0, 4); bc_unit(B0, 5)
                qk_tile(c, *qkt[9])
                sc_unit(B1, 5); av_unit(B1, 3)
                recip_unit(B1, 0)
                op_unit(B0, 0)
                qk_tile(c, *qkt[10])
                av_unit(B1, 4)
                op_unit(B0, 1)
                qk_tile(c, *qkt[11])
                av_unit(B1, 5)
                recip_unit(B1, 1)
                vp_unit(C0, 0); vp_unit(C0, 1)
                bc_unit(B1, 0); bc_unit(B1, 1); bc_unit(B1, 2)
                vp_unit(C0, 2); vp_unit(C0, 3)
                bc_unit(B1, 3); bc_unit(B1, 4); bc_unit(B1, 5)
                vp_unit(C1, 0); vp_unit(C1, 1)
                op_unit(B1, 0)
                vp_unit(C1, 2); vp_unit(C1, 3)
                op_unit(B1, 1)

            # iteration 3 + tail, fully merged: att(pair 2) woven into
            # chunk 3, with pair-3 attention layered in as soon as each
            # head-pair's q/k tiles land, so the PE stays dense to the end.
            # (sc(6/7, n) needs chunk-3 tiles q_n AND k_n = qkt[2n+1];
            # e_t ring backlog must stay <= bufs-2 to avoid WAR cycles.)
            sc_unit(4, 0); sc_unit(4, 1)
            qk_tile(3, *qkt[0])
            sc_unit(4, 2); av_unit(4, 0)
            qk_tile(3, *qkt[1])
            sc_unit(4, 3); av_unit(4, 1)
            qk_tile(3, *qkt[2])
            sc_unit(4, 4); av_unit(4, 2)
            qk_tile(3, *qkt[3])
            sc_unit(4, 5); av_unit(4, 3)
            recip_unit(4, 0)
            qk_tile(3, *qkt[4])
            av_unit(4, 4); av_unit(4, 5)
            recip_unit(4, 1)
            qk_tile(3, *qkt[5])
            sc_unit(5, 0); sc_unit(5, 1)
            qk_tile(3, *qkt[6])
            sc_unit(5, 2); av_unit(5, 0); sc_unit(6, 0)
            bc_unit(4, 0); bc_unit(4, 1); bc_unit(4, 2)
            qk_tile(3, *qkt[7])
            sc_unit(5, 3); av_unit(5, 1); sc_unit(6, 1)
            bc_unit(4, 3); bc_unit(4, 4); bc_unit(4, 5)
            vp_unit(6, 0); vp_unit(6, 1)
            qk_tile(3, *qkt[8])
            sc_unit(5, 4); av_unit(5, 2); sc_unit(6, 2)
            vp_unit(6, 2); vp_unit(6, 3)
            qk_tile(3, *qkt[9])
            sc_unit(5, 5); av_unit(5, 3); av_unit(6, 0)
            recip_unit(5, 0)
            op_unit(4, 0)
            qk_tile(3, *qkt[10])
            av_unit(5, 4); av_unit(5, 5); av_unit(6, 1)
            recip_unit(5, 1)
            qk_tile(3, *qkt[11])
            sc_unit(6, 3); sc_unit(6, 4)
            vp_unit(7, 0); vp_unit(7, 1); vp_unit(7, 2); vp_unit(7, 3)
            bc_unit(5, 0); bc_unit(5, 1); bc_unit(5, 2)
            av_unit(6, 2); sc_unit(6, 5)
            op_unit(4, 1)
            bc_unit(5, 3); bc_unit(5, 4); bc_unit(5, 5)
            av_unit(6, 3); sc_unit(7, 0)
            recip_unit(6, 0)
            op_unit(5, 0)
            av_unit(6, 4); sc_unit(7, 1)
            op_unit(5, 1)
            av_unit(6, 5); sc_unit(7, 2)
            recip_unit(6, 1)
            av_unit(7, 0); sc_unit(7, 3)
            bc_unit(6, 0); bc_unit(6, 1); bc_unit(6, 2)
            av_unit(7, 1); sc_unit(7, 4)
            bc_unit(6, 3); bc_unit(6, 4); bc_unit(6, 5)
            av_unit(7, 2); sc_unit(7, 5)
            av_unit(7, 3)
            recip_unit(7, 0)
            op_unit(6, 0)
            av_unit(7, 4); av_unit(7, 5)
            recip_unit(7, 1)
            op_unit(6, 1)
            bc_unit(7, 0); bc_unit(7, 1); bc_unit(7, 2)
            bc_unit(7, 3); bc_unit(7, 4); bc_unit(7, 5)
            op_unit(7, 0)
            op_unit(7, 1)

    nc.compile()
    return nc


def _split16(a):
    hi = a.astype(np.float16)
    lo = (a - hi.astype(np.float32)).astype(np.float16)
    return hi, lo


def _pack_w(a16):
    # [768, N] fp16 -> [128, 6*N]: out[p, k*N+c] = a16[k*128+p, c]
    N = a16.shape[1]
    return np.ascontiguousarray(
        a16.reshape(ND, 128, N).transpose(1, 0, 2).reshape(128, ND * N))


def _prep_weights(Wq, bq, Wk, bk, Wv, bv, Wo, bo, has_bias):
    f32 = np.float32
    wq = np.asarray(Wq, f32) * f32(0.125)
    wk = np.asarray(Wk, f32)
    wq_hi, wq_lo = _split16(wq)
    wk_hi, wk_lo = _split16(wk)
    w = {
        "wq_hi": _pack_w(wq_hi),
        "wk_hi": _pack_w(wk_hi),
        "wv": _pack_w(np.asarray(Wv, f32).astype(np.float16)),
        "wo": _pack_w(np.asarray(Wo, f32).astype(np.float16)),
    }
    if N_TERMS >= 2:
        w["wq_lo"] = _pack_w(wq_lo)
        w["wk_lo"] = _pack_w(wk_lo)
    if has_bias:
        w["bq"] = (np.asarray(bq, f32) * f32(0.125)).astype(
            np.float16).reshape(1, D)
        w["bk"] = np.asarray(bk, f32).astype(np.float16).reshape(1, D)
        w["bv"] = np.asarray(bv, f32).astype(np.float16).reshape(1, D)
        w["bo"] = np.asarray(bo, f32).astype(np.float16).reshape(1, D)
    return w


def _pack_x(xc16):
    # [R, 768] fp16 -> [128, 6*RPAD]: out[p, k*RPAD+j] = x16[j, k*128+p]
    xt = np.zeros((D, RPAD), np.float16)
    xt[:, :R] = xc16.T
    return np.ascontiguousarray(
        xt.reshape(ND, 128, RPAD).transpose(1, 0, 2).reshape(128, ND * RPAD))


def _make_in_maps(x, w):
    x = np.asarray(x, np.float32)
    in_maps = []
    for c in range(NCORES):
        m = dict(w)
        xc = x[c * BL:(c + 1) * BL].reshape(R, D)
        x16 = xc.astype(np.float16)
        m["x16p"] = _pack_x(x16)
        if N_TERMS == 3:
            m["xlop"] = _pack_x(
                (xc - x16.astype(np.float32)).astype(np.float16))
        in_maps.append(m)
    return in_maps


def kernel(x, Wq, bq, Wk, bk, Wv, bv, Wo, bo):
    from concourse import bass_utils

    has_bias = any(float(np.abs(np.asarray(v)).max()) != 0.0
                   for v in (bq, bk, bv, bo))
    key = ("nc", has_bias, N_TERMS)
    if key not in _CACHE:
        _CACHE[key] = _build(has_bias, N_TERMS)
    nc = _CACHE[key]

    w = _prep_weights(Wq, bq, Wk, bk, Wv, bv, Wo, bo, has_bias)
    in_maps = _make_in_maps(x, w)

    res = bass_utils.run_bass_kernel_spmd(nc, in_maps, list(range(NCORES)))
    out = np.concatenate(
        [res.results[c]["out"].reshape(BL, T, D) for c in range(NCORES)],
        axis=0)
    return out.astype(np.float32)


# revision 44
# speedup vs baseline: 1.0797x; 1.0534x over previous
"""Multi-head attention forward on 8 TRN2 NeuronCores (data-parallel over batch).

Reference computation (B=64, T=197, D=768, H=12, DK=64, fp32):
    q = split_heads(x @ Wq + bq); k = ...; v = ...
    scores = floor((q @ k^T) / 8); attn = softmax(scores); out = attn @ v
    return merge_heads(out) @ Wo + bo

Numerics: floor() before softmax makes the Q/K path sensitive.  q/k
projections run as plain fp16 matmuls (N_TERMS=1) with exact fp32
PSUM accumulation.  Measured rel err vs the fp32 reference: 1.657e-2
(budget 2e-2) — bitwise-reproducible across runs (deterministic
inputs, deterministic PE accumulation order) and equal to the numpy
simulation of the same scheme, so the margin is not subject to
run-to-run noise.  N_TERMS=2 (W split hi+lo: + x16 @ W_lo, rel err
1.48e-2, +22us) and N_TERMS=3 (+ xlo @ W_hi, rel err 1.2e-3, +45us)
are kept as fallbacks.  The scores matmul is native fp32, two heads
row-packed via tile_position (packed pairs execute concurrently on
the PE).  The V path (v proj, attn@v, out proj) runs in plain fp16.

Layout: x is transposed + fp16-cast + partition-packed on the HOST
(x16p[p, k*1584+j] = x16[j, k*128+p]) and weights partition-packed
(w[p, k*768+c] = W[k*128+p, c]) so every DMA descriptor is a 9-19KB
contiguous run — the on-chip transpose phase and its 256B-descriptor
DMA-transpose storm are gone entirely (input DMA ~7us, was ~50us).

Schedule: one software-pipelined instruction stream so the PE never
idles (idle >3.4us re-throttles the PE clock to 1.2GHz via HAM).
Projections run in 4 column chunks of 394 rows = one batch pair each.
Attention for pair p (scores -> floor (DVE magic-number round) -> Exp
(ScalarE, -MAGIC folded into the bias) -> attn@v -> normalize -> out
proj) is woven into projection chunk p+1's matmul stream; v-proj units
(dependency-free) pad the latency-sensitive spots (reciprocal ->
broadcast).  Pair 3's attention is layered into iteration 3 as soon
as each head-pair's q/k tiles land, so the tail is short.  attn@v
right-appends a ones column per head (v stride 65) so the softmax
denominator falls out of the same matmul; denominators are gathered
at partitions 0/32 (col block = head pair), reciprocal in two batched
halves (after hp3 / after hp5), then ONE K=33 selector matmul per
head pair broadcasts both reciprocal rows (out rows 0-63 <- partition
0, rows 64-127 <- partition 32; the dn ring buffers are pre-filled
with 1.0 so selector-weight-0 rows stream finite values), DVE
multiply.  PE warmup matmuls + Exp-table preload run during the
initial DMA wait.  PSUM: main 6-bank ring + 2-bank broadcast ring
(decouples broadcasts from Scalar-freed main-ring tenants).

Ring-buffer backlogs (e_t etc.) are sized so WAR reuse never creates
a cross-engine semaphore cycle: an exp writing e_t slot N+16 waits on
the attn@v matmuls of slot N, which must already be issued.
"""

import numpy as np

B, T, D, H, DK = 64, 197, 768, 12, 64
NCORES = 8
BL = B // NCORES          # 8 batch elements per core
R = BL * T                # 1576 rows per core
RPAD = 1584               # row count padded (keeps host packing regular)
ND = D // 128             # 6 chunks of 128 along D
NC4 = 4                   # proj col chunks (one batch pair each)
CW = R // NC4             # 394 = 2*T
HV = DK + 1               # 65: per-head v stride (ones column at 64)
KEYCHUNKS = [(0, 128), (128, 69)]
MAGIC = float(3 * 2 ** 22)  # 1.5*2^23: x-0.5+MAGIC stays in [2^23,2^24), ulp=1
N_TERMS = 1               # 1: q/k = x16@W16; 2: W split hi+lo; 3: + xlo@W_hi

_CACHE = {}


def _build(has_bias, n_terms):
    import concourse.bacc as bacc
    import concourse.mybir as mybir
    import concourse.tile as tile

    f32 = mybir.dt.float32
    f16 = mybir.dt.float16
    AF = mybir.ActivationFunctionType
    OP = mybir.AluOpType

    nc = bacc.Bacc("TRN2", target_bir_lowering=False, debug=False,
                   num_devices=NCORES)

    x16_d = nc.dram_tensor("x16p", [128, ND * RPAD], f16,
                           kind="ExternalInput").ap()
    if n_terms == 3:
        xlo_d = nc.dram_tensor("xlop", [128, ND * RPAD], f16,
                               kind="ExternalInput").ap()
    wnames = (("wq_hi", "wk_hi", "wv", "wo") if n_terms == 1 else
              ("wq_hi", "wq_lo", "wk_hi", "wk_lo", "wv", "wo"))
    w_d = {}
    for nm in wnames:
        w_d[nm] = nc.dram_tensor(nm, [128, ND * D], f16,
                                 kind="ExternalInput").ap()
    if has_bias:
        b_d = {nm: nc.dram_tensor(nm, [1, D], f16, kind="ExternalInput").ap()
               for nm in ("bq", "bk", "bv", "bo")}
    out_d = nc.dram_tensor("out", [R, D], f32, kind="ExternalOutput").ap()

    with tile.TileContext(nc) as tc:
        with tc.tile_pool(name="static", bufs=1) as Ps, \
             tc.tile_pool(name="work", bufs=1) as Pw, \
             tc.tile_pool(name="psum", bufs=8, space="PSUM") as Pp:

            def ptile(nm):
                return Pp.tile([128, CW], f32, name=nm, tag="ps", bufs=6,
                               uniquify=True)

            def bctile():
                # own two-bank ring: broadcasts then only wait on their
                # own previous consumers (DVE mult), not main-ring tenants
                return Pp.tile([128, CW], f32, name="bc", tag="bcp", bufs=2,
                               uniquify=True)

            xall = Ps.tile([128, ND * RPAD], f16, name="xall")
            if n_terms == 3:
                xloall = Ps.tile([128, ND * RPAD], f16, name="xloall")
            wsb = {nm: Ps.tile([128, ND * D], f16, name=nm) for nm in wnames}
            # v16e[2b+kc][keys<=128, 12*65]; col h*65+64 holds ones
            v16e = [Ps.tile([128, H * HV], f16, name=f"v16e_{i}")
                    for i in range(2 * BL)]
            ones_row = Ps.tile([128, CW], f16, name="ones_row")
            negmagic = Ps.tile([128, 1], f32, name="negmagic")
            prime = Ps.tile([1, 1], f16, name="prime")
            # broadcast selector: out rows 0-63 <- rhs row 0 (partition pb),
            # rows 64-127 <- rhs row 32 (partition pb+32), in ONE K=33 matmul
            sel = Ps.tile([128, 128], f16, name="sel")
            if has_bias:
                bsb = {nm: Ps.tile([1, D], f16, name=f"{nm}_sb")
                       for nm in ("bq", "bk", "bv", "bo")}

            def xs(k, c0, ln):
                return xall[:, k * RPAD + c0:k * RPAD + c0 + ln]

            def xls(k, c0, ln):
                return xloall[:, k * RPAD + c0:k * RPAD + c0 + ln]

            def ws(nm, k, c0, ln):
                return wsb[nm][:, k * D + c0:k * D + c0 + ln]

            # ---- no-DMA-dependency setup: memsets, engine warmups ----
            nc.vector.memset(ones_row, 1.0)
            nc.vector.memset(negmagic, -MAGIC)
            nc.vector.memset(sel, 0.0)
            nc.vector.memset(sel[0:1, 0:64], 1.0)
            nc.vector.memset(sel[32:33, 64:128], 1.0)
            # pre-fill the dn ring buffers with 1.0: the K=33 broadcast
            # matmul streams rows 1-31 (selector weight 0) — they must be
            # finite or 0*inf => NaN.  Ring reuse preserves the fill since
            # dn copies only ever write rows 0 and 32.
            for i in range(3):
                dmy = Pw.tile([128, 4 * T], f32, name="dnA", tag="dnA",
                              bufs=3, uniquify=True)
                nc.vector.memset(dmy[:33, :], 1.0)
                dmy = Pw.tile([128, 2 * T], f32, name="dnB", tag="dnB",
                              bufs=3, uniquify=True)
                nc.vector.memset(dmy[:33, :], 1.0)
            for i in range(2 * BL):
                onescol = v16e[i].rearrange("p (h c) -> p h c",
                                            c=HV)[:, :, DK:DK + 1]
                nc.gpsimd.memset(onescol, 1.0)
            # Exp table preload on ScalarE (one-time 1.3us table load)
            nc.scalar.activation(prime, ones_row[:1, :1], AF.Exp,
                                 bias=negmagic[:1, :1])
            # PE warmup: keep HAM at full clock until real work arrives
            for i in range(30):
                wu = ptile("wu")
                nc.tensor.matmul(wu, ones_row[:, :128], ones_row,
                                 start=True, stop=True)

            # ---- DMAs (all large contiguous descriptors) ----
            # x split by column range in consumption order: chunk 0
            # (cols < 400) gates the first projections on just 0.62MB
            x3 = xall.rearrange("p (k j) -> p k j", k=ND)
            xd3 = x16_d.rearrange("p (k j) -> p k j", k=ND)
            nc.sync.dma_start(x3[:, :, :400], xd3[:, :, :400])
            nc.sync.dma_start(wsb["wq_hi"], w_d["wq_hi"])
            if n_terms >= 2:
                nc.sync.dma_start(wsb["wq_lo"], w_d["wq_lo"])
            nc.sync.dma_start(x3[:, :, 400:800], xd3[:, :, 400:800])
            nc.sync.dma_start(wsb["wk_hi"], w_d["wk_hi"])
            if n_terms >= 2:
                nc.sync.dma_start(wsb["wk_lo"], w_d["wk_lo"])
            nc.sync.dma_start(x3[:, :, 800:], xd3[:, :, 800:])
            if n_terms == 3:
                nc.sync.dma_start(xloall, xlo_d)
            nc.sync.dma_start(wsb["wv"], w_d["wv"])
            nc.sync.dma_start(wsb["wo"], w_d["wo"])
            if has_bias:
                for nm in ("bq", "bk", "bv", "bo"):
                    nc.sync.dma_start(bsb[nm], b_d[nm])

            # ---- stage helpers (each call ISSUES instructions) ----
            qT = {}   # (proj, c, n) -> sbuf tile [128, CW] f32
            eTs = {}  # (b, hp) -> [e_t hl0, e_t hl1]
            otfs = {}  # (b, hp) -> otf tile
            oT16s = {}  # (b, hp) -> oT16 tile
            dns = {}
            rd16s = {}

            def qk_tile(c, proj, n):
                whi, wlo, b_nm = (("wq_hi", "wq_lo", "bq") if proj == "q"
                                  else ("wk_hi", "wk_lo", "bk"))
                c0 = c * CW
                pp = ptile("pp")
                for k in range(ND):
                    last = (k == ND - 1 and n_terms == 1 and not has_bias)
                    nc.tensor.matmul(pp, ws(whi, k, n * 128, 128),
                                     xs(k, c0, CW), start=(k == 0),
                                     stop=last)
                for k in range(ND if n_terms >= 2 else 0):
                    last = (k == ND - 1 and n_terms == 2 and not has_bias)
                    nc.tensor.matmul(pp, ws(wlo, k, n * 128, 128),
                                     xs(k, c0, CW), start=False, stop=last)
                if n_terms == 3:
                    for k in range(ND):
                        last = (k == ND - 1 and not has_bias)
                        nc.tensor.matmul(pp, ws(whi, k, n * 128, 128),
                                         xls(k, c0, CW),
                                         start=False, stop=last)
                if has_bias:
                    nc.tensor.matmul(pp, bsb[b_nm][:1, n * 128:n * 128 + 128],
                                     ones_row[:1, :CW],
                                     start=False, stop=True)
                dst = Pw.tile([128, CW], f32, name=f"{proj}T", tag=f"{proj}T",
                              bufs=12, uniquify=True)
                nc.scalar.activation(dst, pp, AF.Copy)
                qT[(proj, c, n)] = dst

            def vp_unit(b, j):
                kc, half = j // 2, j % 2
                koff, klen = KEYCHUNKS[kc]
                base = b * T
                c0 = half * 384
                vp = ptile("vp")
                vps = vp[:klen, :384]
                for d in range(ND):
                    nc.tensor.matmul(
                        vps, xs(d, base + koff, klen), ws("wv", d, c0, 384),
                        start=(d == 0),
                        stop=(d == ND - 1 and not has_bias))
                if has_bias:
                    nc.tensor.matmul(vps, ones_row[:1, :klen],
                                     bsb["bv"][:1, c0:c0 + 384],
                                     start=False, stop=True)
                dst = v16e[2 * b + kc]
                dst3 = dst[:klen, :].rearrange("p (h c) -> p h c",
                                               c=HV)[:, :, 0:DK]
                nc.scalar.activation(
                    dst3[:, half * 6:(half + 1) * 6, :],
                    vps.rearrange("p (h c) -> p h c", c=DK), AF.Copy)

            def sc_unit(b, hp):
                c = b // 2
                qoff = (b % 2) * T
                eT = []
                for hl in range(2):
                    pb = 64 * hl
                    sc = ptile("sc")
                    for kc, (koff, klen) in enumerate(KEYCHUNKS):
                        nc.tensor.matmul(
                            sc[:klen, kc * T:(kc + 1) * T],
                            qT[("k", c, hp)][pb:pb + 64,
                                             qoff + koff:qoff + koff + klen],
                            qT[("q", c, hp)][pb:pb + 64, qoff:qoff + T],
                            start=True, stop=True, tile_position=(pb, 0))
                    fl = Pw.tile([128, 2 * T], f32, name="fl", tag="fl",
                                 bufs=5, uniquify=True)
                    nc.vector.tensor_scalar(fl, sc, -0.5, MAGIC,
                                            OP.add, OP.add)
                    e_t = Pw.tile([128, 2 * T], f16, name="e_t", tag="eT",
                                  bufs=16, uniquify=True)
                    nc.scalar.activation(e_t, fl, AF.Exp,
                                         bias=negmagic[:, :1])
                    eT.append(e_t)
                eTs[(b, hp)] = eT

            def av_unit(b, hp):
                eT = eTs.pop((b, hp))
                if hp == 0:
                    # denominator gather split in two so the reciprocal can
                    # start after hp 0-3 (heads 0-7) instead of after all 12.
                    # head pair hp lands at partitions (0, 32), col block hp
                    dns[b] = (
                        Pw.tile([128, 4 * T], f32, name="dnA", tag="dnA",
                                bufs=3, uniquify=True),
                        Pw.tile([128, 2 * T], f32, name="dnB", tag="dnB",
                                bufs=3, uniquify=True))
                dnA, dnB = dns[b]
                otf = Pw.tile([128, T], f32, name="otf", tag="otf",
                              bufs=12, uniquify=True)
                op_ = ptile("oT")
                for hl in range(2):
                    h = 2 * hp + hl
                    for kc, (koff, klen) in enumerate(KEYCHUNKS):
                        nc.tensor.matmul(
                            op_[0:HV, hl * T:(hl + 1) * T],
                            v16e[2 * b + kc][:klen, h * HV:(h + 1) * HV],
                            eT[hl][:klen, kc * T:(kc + 1) * T],
                            start=(kc == 0), stop=(kc == len(KEYCHUNKS) - 1))
                # both denominator copies first: they feed the latency-
                # critical recip->broadcast chain; otf staging can lag.
                # tail pairs route to ScalarE to keep the Vector queue clear
                dn, cb = (dnA, hp * T) if hp < 4 else (dnB, (hp - 4) * T)
                for hl in range(2):
                    pbase = 32 * hl
                    if b >= 6:
                        nc.scalar.activation(
                            dn[pbase:pbase + 1, cb:cb + T],
                            op_[64:65, hl * T:(hl + 1) * T], AF.Copy)
                    else:
                        nc.vector.tensor_copy(
                            dn[pbase:pbase + 1, cb:cb + T],
                            op_[64:65, hl * T:(hl + 1) * T])
                nc.scalar.activation(otf[0:64, :], op_[0:64, :T], AF.Copy)
                if b >= 6:
                    nc.scalar.activation(otf[64:128, :],
                                         op_[0:64, T:2 * T], AF.Copy)
                else:
                    nc.vector.tensor_copy(otf[64:128, :],
                                          op_[0:64, T:2 * T])
                otfs[(b, hp)] = otf

            def recip_unit(b, part):
                dnA, dnB = dns[b]
                # the fp16 cast runs on idle GpSimd in steady state, but on
                # DVE for the tail pair where the rd16->bcast latency matters
                cp = nc.vector.tensor_copy if b >= 6 else nc.gpsimd.tensor_copy
                if part == 0:
                    rdf = Pw.tile([128, 4 * T], f32, name="rdfA", tag="rdfA",
                                  bufs=1, uniquify=True)
                    rd16 = Pw.tile([128, 4 * T], f16, name="rdA", tag="rdA",
                                   bufs=3, uniquify=True)
                    nc.vector.reciprocal_approx_fast(rdf[:33, :],
                                                     dnA[:33, :])
                    cp(rd16[:33, :], rdf[:33, :])
                    rd16s[b] = [rd16, None]
                else:
                    rdf = Pw.tile([128, 2 * T], f32, name="rdfB", tag="rdfB",
                                  bufs=1, uniquify=True)
                    rd16 = Pw.tile([128, 2 * T], f16, name="rdB", tag="rdB",
                                   bufs=3, uniquify=True)
                    nc.vector.reciprocal_approx_fast(rdf[:33, :],
                                                     dnB[:33, :])
                    cp(rd16[:33, :], rdf[:33, :])
                    rd16s[b][1] = rd16
                    del dns[b]

            def bc_unit(b, hp):
                bc = bctile()
                rd16, cb = ((rd16s[b][0], hp * T) if hp < 4
                            else (rd16s[b][1], (hp - 4) * T))
                nc.tensor.matmul(
                    bc[:, :T], sel[0:33, :128], rd16[0:33, cb:cb + T],
                    start=True, stop=True)
                oT16 = Pw.tile([128, T], f16, name="oT16", tag="oT16",
                               bufs=14, uniquify=True)
                nc.vector.tensor_tensor(oT16, otfs.pop((b, hp)), bc[:, :T],
                                        OP.mult)
                oT16s[(b, hp)] = oT16

            def op_unit(b, rc):
                base = b * T
                roff, rn = (0, 128) if rc == 0 else (128, T - 128)
                fs = Pw.tile([128, D], f32, name="fs", tag="fs",
                             bufs=3, uniquify=True)
                for half in range(2):
                    c0 = half * 384
                    fp_ = ptile("fp")
                    for d in range(ND):
                        nc.tensor.matmul(
                            fp_[:rn, :384],
                            oT16s[(b, d)][:, roff:roff + rn],
                            ws("wo", d, c0, 384),
                            start=(d == 0),
                            stop=(d == ND - 1 and not has_bias))
                    if has_bias:
                        nc.tensor.matmul(fp_[:rn, :384], ones_row[:1, :rn],
                                         bsb["bo"][:1, c0:c0 + 384],
                                         start=False, stop=True)
                    nc.scalar.activation(fs[:rn, c0:c0 + 384],
                                         fp_[:rn, :384], AF.Copy)
                    nc.sync.dma_start(
                        out_d[base + roff:base + roff + rn, c0:c0 + 384],
                        fs[:rn, c0:c0 + 384])
                if rc == 1:
                    for d in range(ND):
                        del oT16s[(b, d)]

            # ---- pipelined program ----
            # c0 prologue: projections chunk 0 + v for pair 0
            for n in range(ND):
                qk_tile(0, "q", n)
            for n in range(ND):
                qk_tile(0, "k", n)
            for j in range(4):
                vp_unit(0, j)
            for j in range(4):
                vp_unit(1, j)

            # iterations c=1,2: attention(pair c-1) woven into chunk c
            # interleaved q/k tile order: after tiles 2n,2n+1 the score
            # units for head-pair n of the NEXT pair are unblocked
            qkt = [(p, n) for n in range(ND) for p in ("q", "k")]
            for c in range(1, 3):
                B0, B1 = 2 * (c - 1), 2 * (c - 1) + 1
                C0, C1 = 2 * c, 2 * c + 1

                sc_unit(B0, 0); sc_unit(B0, 1)
                qk_tile(c, *qkt[0])
                sc_unit(B0, 2); av_unit(B0, 0)
                qk_tile(c, *qkt[1])
                sc_unit(B0, 3); av_unit(B0, 1)
                qk_tile(c, *qkt[2])
                sc_unit(B0, 4); av_unit(B0, 2)
                qk_tile(c, *qkt[3])
                sc_unit(B0, 5); av_unit(B0, 3)
                recip_unit(B0, 0)
                qk_tile(c, *qkt[4])
                av_unit(B0, 4); av_unit(B0, 5)
                recip_unit(B0, 1)
                qk_tile(c, *qkt[5])
                sc_unit(B1, 0); sc_unit(B1, 1)
                qk_tile(c, *qkt[6])
                sc_unit(B1, 2); av_unit(B1, 0)
                bc_unit(B0, 0); bc_unit(B0, 1)
                qk_tile(c, *qkt[7])
                sc_unit(B1, 3); av_unit(B1, 1)
                bc_unit(B0, 2); bc_unit(B0, 3)
                qk_tile(c, *qkt[8])
                sc_unit(B1, 4); av_unit(B1, 2)
                bc_unit(B0, 4); bc_unit(B0, 5)
                qk_tile(c, *qkt[9])
                sc_unit(B1, 5); av_unit(B1, 3)
                recip_unit(B1, 0)
                op_unit(B0, 0)
                qk_tile(c, *qkt[10])
                av_unit(B1, 4)
                op_unit(B0, 1)
                qk_tile(c, *qkt[11])
                av_unit(B1, 5)
                recip_unit(B1, 1)
                vp_unit(C0, 0); vp_unit(C0, 1)
                bc_unit(B1, 0); bc_unit(B1, 1); bc_unit(B1, 2)
                vp_unit(C0, 2); vp_unit(C0, 3)
                bc_unit(B1, 3); bc_unit(B1, 4); bc_unit(B1, 5)
                vp_unit(C1, 0); vp_unit(C1, 1)
                op_unit(B1, 0)
                vp_unit(C1, 2); vp_unit(C1, 3)
                op_unit(B1, 1)

            # iteration 3 + tail, fully merged: att(pair 2) woven into
            # chunk 3, with pair-3 attention layered in as soon as each
            # head-pair's q/k tiles land, so the PE stays dense to the end.
            # (sc(6/7, n) needs chunk-3 tiles q_n AND k_n = qkt[2n+1];
            # e_t ring backlog must stay <= bufs-2 to avoid WAR cycles.)
            sc_unit(4, 0); sc_unit(4, 1)
            qk_tile(3, *qkt[0])
            sc_unit(4, 2); av_unit(4, 0)
            qk_tile(3, *qkt[1])
            sc_unit(4, 3); av_unit(4, 1)
            qk_tile(3, *qkt[2])
            sc_unit(4, 4); av_unit(4, 2)
            qk_tile(3, *qkt[3])
            sc_unit(4, 5); av_unit(4, 3)
            recip_unit(4, 0)
            qk_tile(3, *qkt[4])
            av_unit(4, 4); av_unit(4, 5)
            recip_unit(4, 1)
            qk_tile(3, *qkt[5])
            sc_unit(5, 0); sc_unit(5, 1)
            qk_tile(3, *qkt[6])
            sc_unit(5, 2); av_unit(5, 0); sc_unit(6, 0)
            bc_unit(4, 0); bc_unit(4, 1); bc_unit(4, 2)
            qk_tile(3, *qkt[7])
            sc_unit(5, 3); av_unit(5, 1); sc_unit(6, 1)
            bc_unit(4, 3); bc_unit(4, 4); bc_unit(4, 5)
            vp_unit(6, 0); vp_unit(6, 1)
            qk_tile(3, *qkt[8])
            sc_unit(5, 4); av_unit(5, 2); sc_unit(6, 2)
            vp_unit(6, 2); vp_unit(6, 3)
            qk_tile(3, *qkt[9])
            sc_unit(5, 5); av_unit(5, 3); av_unit(6, 0)
            recip_unit(5, 0)
            op_unit(4, 0)
            qk_tile(3, *qkt[10])
            av_unit(5, 4); av_unit(5, 5); av_unit(6, 1)
            recip_unit(5, 1)
            qk_tile(3, *qkt[11])
            sc_unit(6, 3); sc_unit(6, 4)
            vp_unit(7, 0); vp_unit(7, 1); vp_unit(7, 2); vp_unit(7, 3)
            bc_unit(5, 0); bc_unit(5, 1); bc_unit(5, 2)
            av_unit(6, 2); sc_unit(6, 5)
            op_unit(4, 1)
            bc_unit(5, 3); bc_unit(5, 4); bc_unit(5, 5)
            av_unit(6, 3); sc_unit(7, 0)
            recip_unit(6, 0)
            op_unit(5, 0)
            av_unit(6, 4); sc_unit(7, 1)
            op_unit(5, 1)
            av_unit(6, 5); sc_unit(7, 2)
            recip_unit(6, 1)
            av_unit(7, 0); sc_unit(7, 3)
            bc_unit(6, 0); bc_unit(6, 1); bc_unit(6, 2)
            av_unit(7, 1); sc_unit(7, 4)
            bc_unit(6, 3); bc_unit(6, 4); bc_unit(6, 5)
            av_unit(7, 2); sc_unit(7, 5)
            av_unit(7, 3)
            recip_unit(7, 0)
            op_unit(6, 0)
            av_unit(7, 4); av_unit(7, 5)
            recip_unit(7, 1)
            op_unit(6, 1)
            bc_unit(7, 0); bc_unit(7, 1); bc_unit(7, 2)
            bc_unit(7, 3); bc_unit(7, 4); bc_unit(7, 5)
            op_unit(7, 0)
            op_unit(7, 1)

    nc.compile()
    return nc


def _split16(a):
    hi = a.astype(np.float16)
    lo = (a - hi.astype(np.float32)).astype(np.float16)
    return hi, lo


def _pack_w(a16):
    # [768, N] fp16 -> [128, 6*N]: out[p, k*N+c] = a16[k*128+p, c]
    N = a16.shape[1]
    return np.ascontiguousarray(
        a16.reshape(ND, 128, N).transpose(1, 0, 2).reshape(128, ND * N))


def _prep_weights(Wq, bq, Wk, bk, Wv, bv, Wo, bo, has_bias):
    f32 = np.float32
    wq = np.asarray(Wq, f32) * f32(0.125)
    wk = np.asarray(Wk, f32)
    wq_hi, wq_lo = _split16(wq)
    wk_hi, wk_lo = _split16(wk)
    w = {
        "wq_hi": _pack_w(wq_hi),
        "wk_hi": _pack_w(wk_hi),
        "wv": _pack_w(np.asarray(Wv, f32).astype(np.float16)),
        "wo": _pack_w(np.asarray(Wo, f32).astype(np.float16)),
    }
    if N_TERMS >= 2:
        w["wq_lo"] = _pack_w(wq_lo)
        w["wk_lo"] = _pack_w(wk_lo)
    if has_bias:
        w["bq"] = (np.asarray(bq, f32) * f32(0.125)).astype(
            np.float16).reshape(1, D)
        w["bk"] = np.asarray(bk, f32).astype(np.float16).reshape(1, D)
        w["bv"] = np.asarray(bv, f32).astype(np.float16).reshape(1, D)
        w["bo"] = np.asarray(bo, f32).astype(np.float16).reshape(1, D)
    return w


def _pack_x(xc16):
    # [R, 768] fp16 -> [128, 6*RPAD]: out[p, k*RPAD+j] = x16[j, k*128+p]
    xt = np.zeros((D, RPAD), np.float16)
    xt[:, :R] = xc16.T
    return np.ascontiguousarray(
        xt.reshape(ND, 128, RPAD).transpose(1, 0, 2).reshape(128, ND * RPAD))


def _make_in_maps(x, w):
    x = np.asarray(x, np.float32)
    in_maps = []
    for c in range(NCORES):
        m = dict(w)
        xc = x[c * BL:(c + 1) * BL].reshape(R, D)
        x16 = xc.astype(np.float16)
        m["x16p"] = _pack_x(x16)
        if N_TERMS == 3:
            m["xlop"] = _pack_x(
                (xc - x16.astype(np.float32)).astype(np.float16))
        in_maps.append(m)
    return in_maps


def kernel(x, Wq, bq, Wk, bk, Wv, bv, Wo, bo):
    from concourse import bass_utils

    has_bias = any(float(np.abs(np.asarray(v)).max()) != 0.0
                   for v in (bq, bk, bv, bo))
    key = ("nc", has_bias, N_TERMS)
    if key not in _CACHE:
        _CACHE[key] = _build(has_bias, N_TERMS)
    nc = _CACHE[key]

    w = _prep_weights(Wq, bq, Wk, bk, Wv, bv, Wo, bo, has_bias)
    in_maps = _make_in_maps(x, w)

    res = bass_utils.run_bass_kernel_spmd(nc, in_maps, list(range(NCORES)))
    out = np.concatenate(
        [res.results[c]["out"].reshape(BL, T, D) for c in range(NCORES)],
        axis=0)
    return out.astype(np.float32)
